# revision 12
# baseline (speedup 1.0000x reference)
"""Trainium2 Bass kernel for the SRNN adapter problem (hardware-loop version).

Strategy (8 cores, data-parallel over batch B=256 -> 32 per core):
  The dominant cost in this environment is per-UNIQUE-instruction dispatch
  (~40us each, measured); looped re-execution via tc.For_i is ~free. So the
  99-step scan is expressed as hardware loops with dynamic (register-offset)
  access patterns instead of 99 unrolled copies:
    - scan loop: steps 0..98, one step per iteration; vo_t = W_out @ u_t is
      computed EVERY step (8 extra matmuls) into a 16-slot output ring at
      column (t & 15) * BL, which eliminates the snapshot + epilogue loop
      entirely (the host reads slots (89+s) & 15 for the last 10 steps)
  Per scan step (in-loop, all APs static except the xt column offset t*BL):
    psum[h] = sum_dk W_inT[dk,h] @ xT[dk, t-cols]    (input projection)
            + sum_k (W_rec_eff - THR*I)T[k,h] @ z[k] (recurrence)
    v = alpha*v + psum ; z = v > THR ; u = kappa*u + z   (in-place DVE)
  Host: X pre-transposed to [D, T*BL] per core; softmax+mean over the last
  10 steps on host (0.005% of FLOPs).

All matmul arithmetic is fp32 (measured PE fp32 relnorm ~7e-8 vs fp64; the
spiking threshold makes the system chaotic, so bf16/fp32r decorrelate the
spike trains and blow the error budget).
"""

import sys

sys.path.insert(0, "/opt/trn_rl_repo")

import numpy as np
from contextlib import ExitStack

from concourse import bacc, bass, mybir, tile
from concourse.bass import ds
from concourse.bass_utils import run_bass_kernel_spmd

F32 = mybir.dt.float32
A = mybir.AluOpType

B, T, D, H, O = 256, 100, 700, 1024, 20
NCORES = 8
BL = B // NCORES  # 32 batch rows per core
KT = H // 128  # 8 k/h tiles
DTILES = 6  # ceil(700/128), last tile has 60 rows
DLAST = D - 5 * 128  # 60
NSTEPS = T - 1  # 99 scan steps
NTAIL = 10  # last-K softmax window
XCOLS = NSTEPS * BL  # 3168 transposed-X columns actually used

ALPHA = float(np.float32(np.exp(-1.0 / 20.0)))
KAPPA = float(np.float32(np.exp(-1.0 / 20.0)))
THR = 1.0


WI_OFF = 0
WI_LEN = DTILES * H  # 6144
W_OFF = WI_OFF + WI_LEN
W_LEN = KT * H  # 8192
WO_OFF = W_OFF + W_LEN
WO_LEN = KT * O  # 160
XT_OFF = WO_OFF + WO_LEN  # 14496
XT_LEN = DTILES * XCOLS  # 19008
BLOB_COLS = XT_OFF + XT_LEN  # 33504


def _build(nsteps=NSTEPS, timing=False):
    """timing=True pins the xt column offset to 0 so nsteps can exceed the
    real 99 without reading out of bounds (identical instruction mix)."""
    nc = bacc.Bacc(None)
    blob_d = nc.declare_dram_parameter("blob", [128, BLOB_COLS], F32, isOutput=False)
    vo_d = nc.declare_dram_parameter("vo16", [O, 16 * BL], F32, isOutput=True)

    with ExitStack() as ctx:
        tc = ctx.enter_context(tile.TileContext(nc))
        const = ctx.enter_context(tc.tile_pool(name="const", bufs=1))
        pp = ctx.enter_context(tc.tile_pool(name="pp", bufs=1, space="PSUM"))

        blob_sb = const.tile([128, BLOB_COLS], F32, name="blob_sb")
        xt_sb = blob_sb[:, XT_OFF : XT_OFF + XT_LEN].rearrange(
            "p (a c) -> p a c", a=DTILES
        )
        wi_sb = blob_sb[:, WI_OFF : WI_OFF + WI_LEN].rearrange(
            "p (a c) -> p a c", a=DTILES
        )
        w_sb = blob_sb[:, W_OFF : W_OFF + W_LEN].rearrange("p (a c) -> p a c", a=KT)
        wo_sb = blob_sb[:, WO_OFF : WO_OFF + WO_LEN].rearrange(
            "p (a c) -> p a c", a=KT
        )
        v = const.tile([128, KT, BL], F32, name="v")
        z = const.tile([128, KT, BL], F32, name="z")
        u = const.tile([128, KT, BL], F32, name="u")
        vo_sb = const.tile([O, 16 * BL], F32, name="vo_sb")

        # banks 0-3: scan psum (bank h//2, cols (h%2)*BL); bank 4: vo epilogue
        ps = pp.tile([128, KT, 512], F32, name="ps")

        nc.sync.dma_start(blob_sb[:], blob_d[:])
        nc.vector.memset(v[:], 0.0)
        nc.vector.memset(z[:], 0.0)
        nc.vector.memset(u[:], 0.0)
        nc.vector.memset(vo_sb[:], 0.0)

        if nsteps > 0:
            with tc.For_i(0, nsteps, 1) as t:
                xcol = 0 if timing else t * BL
                for h in range(KT):
                    out = ps[:, h // 2, (h % 2) * BL : (h % 2) * BL + BL]
                    for dk in range(DTILES):
                        w_ = 128 if dk < 5 else DLAST
                        nc.tensor.matmul(
                            out,
                            wi_sb[0:w_, dk, h * 128 : (h + 1) * 128],
                            xt_sb[0:w_, dk, ds(xcol, BL)],
                            start=(dk == 0),
                            stop=False,
                        )
                    for k in range(KT):
                        nc.tensor.matmul(
                            out,
                            w_sb[:, k, h * 128 : (h + 1) * 128],
                            z[:, k, :],
                            start=False,
                            stop=(k == KT - 1),
                        )
                # psum banks 0-3, h-pair (2p, 2p+1) at cols (0,BL) of bank p
                psum_in = ps[:, 0:4, 0 : 2 * BL].rearrange(
                    "q a (b c) -> q a b c", b=2
                )
                v4 = v.rearrange("p (a b) c -> p a b c", a=4)
                nc.vector.scalar_tensor_tensor(
                    v4[:], v4[:], ALPHA, psum_in, A.mult, A.add
                )
                nc.vector.tensor_scalar(z[:], v[:], THR, None, A.is_gt)
                nc.vector.scalar_tensor_tensor(
                    u[:], u[:], KAPPA, z[:], A.mult, A.add
                )
                nc.vector.tensor_copy(usnap[:, :, ds((t & 15) * BL, BL)], u[:])

        # ---- epilogue: vo_s = W_out @ usnap[(slot0 + s) & 15], s = 0..9 ----
        slot0 = max(nsteps - NTAIL, 0) & 15
        with tc.For_i(0, NTAIL, 1) as s:
            vo_ps = ps[0:O, 4, 0:BL]
            for k in range(KT):
                nc.tensor.matmul(
                    vo_ps,
                    wo_sb[:, k, :],
                    usnap[:, k, ds(((s + slot0) & 15) * BL, BL)],
                    start=(k == 0),
                    stop=(k == KT - 1),
                )
            nc.vector.tensor_copy(vo_sb[:, ds(s * BL, BL)], vo_ps)
        nc.gpsimd.dma_start(vo_d[:], vo_sb[:])

    nc.compile()
    return nc


_PROGRAM = None


def _get_program():
    global _PROGRAM
    if _PROGRAM is None:
        _PROGRAM = _build()
    return _PROGRAM


def _host_prep(W_in, W_rec, W_out):
    eye = np.eye(H, dtype=np.float32)
    # z @ w_rec_eff.T - z*THR == z @ (w_rec_eff - THR*eye).T ; lhsT layout [k, h]
    WrT = (W_rec * (1.0 - eye) - np.float32(THR) * eye).T.astype(np.float32)
    WiT = np.zeros((DTILES * 128, H), np.float32)
    WiT[:D] = W_in.T.astype(np.float32)
    WoT = W_out.T.astype(np.float32)  # [H, O]
    # weight section of the blob, identical for every core: [128, cols]
    wpart = np.concatenate(
        [
            WiT.reshape(DTILES, 128, H).transpose(1, 0, 2).reshape(128, -1),
            WrT.reshape(KT, 128, H).transpose(1, 0, 2).reshape(128, -1),
            WoT.reshape(KT, 128, O).transpose(1, 0, 2).reshape(128, -1),
        ],
        axis=1,
    )
    return np.ascontiguousarray(wpart)


def kernel(X, W_in, W_rec, W_out):
    X = np.asarray(X, np.float32)
    wpart = _host_prep(
        np.asarray(W_in, np.float32), np.asarray(W_rec, np.float32),
        np.asarray(W_out, np.float32),
    )
    nc = _get_program()
    in_maps = []
    for c in range(NCORES):
        Xc = X[c * BL : (c + 1) * BL]  # [BL, T, D]
        # [D, t*BL + b] for t = 0..98 (step t uses cols t*BL:(t+1)*BL)
        XTc = np.zeros((DTILES * 128, XCOLS), np.float32)
        XTc[:D] = Xc[:, :NSTEPS, :].transpose(2, 1, 0).reshape(D, XCOLS)
        blob = np.concatenate(
            [wpart,
             XTc.reshape(DTILES, 128, XCOLS).transpose(1, 0, 2).reshape(128, -1)],
            axis=1,
        )
        in_maps.append({"blob": np.ascontiguousarray(blob)})
    res = run_bass_kernel_spmd(nc, in_maps, list(range(NCORES)))
    # vo10 per core: [O, s*BL + b] for scan steps s+89 (vo_full indices 90..99)
    vo = np.stack([r["vo10"] for r in res.results])  # [8, O, 10*BL]
    vo = vo.reshape(NCORES, O, NTAIL, BL).transpose(2, 0, 3, 1).reshape(NTAIL, B, O)
    m = vo.max(axis=2, keepdims=True)
    e = np.exp(vo - m)
    yo = e / e.sum(axis=2, keepdims=True)
    return yo.mean(axis=0).astype(np.float32)


# revision 14
# speedup vs baseline: 61.6382x; 61.6382x over previous
"""Trainium2 Bass kernel for the SRNN adapter problem (hardware-loop version).

Strategy (8 cores, data-parallel over batch B=256 -> 32 per core):
  The dominant cost in this environment is per-UNIQUE-instruction dispatch
  (~40us each, measured); looped re-execution via tc.For_i is ~free. So the
  99-step scan is expressed as hardware loops with dynamic (register-offset)
  access patterns instead of 99 unrolled copies:
    - scan loop: steps 0..98, one step per iteration; vo_t = W_out @ u_t is
      computed EVERY step (8 extra matmuls) into a 16-slot output ring at
      column (t & 15) * BL, which eliminates the snapshot + epilogue loop
      entirely (the host reads slots (89+s) & 15 for the last 10 steps)
  Per scan step (in-loop, all APs static except the xt column offset t*BL):
    psum[h] = sum_dk W_inT[dk,h] @ xT[dk, t-cols]    (input projection)
            + sum_k (W_rec_eff - THR*I)T[k,h] @ z[k] (recurrence)
    v = alpha*v + psum ; z = v > THR ; u = kappa*u + z   (in-place DVE)
  Host: X pre-transposed to [D, T*BL] per core; softmax+mean over the last
  10 steps on host (0.005% of FLOPs).

All matmul arithmetic is fp32 (measured PE fp32 relnorm ~7e-8 vs fp64; the
spiking threshold makes the system chaotic, so bf16/fp32r decorrelate the
spike trains and blow the error budget).
"""

import sys

sys.path.insert(0, "/opt/trn_rl_repo")

import numpy as np
from contextlib import ExitStack

from concourse import bacc, bass, mybir, tile
from concourse.bass import ds
from concourse.bass_utils import run_bass_kernel_spmd

F32 = mybir.dt.float32
A = mybir.AluOpType

B, T, D, H, O = 256, 100, 700, 1024, 20
NCORES = 8
BL = B // NCORES  # 32 batch rows per core
KT = H // 128  # 8 k/h tiles
DTILES = 6  # ceil(700/128), last tile has 60 rows
DLAST = D - 5 * 128  # 60
NSTEPS = T - 1  # 99 scan steps
NTAIL = 10  # last-K softmax window
XCOLS = NSTEPS * BL  # 3168 transposed-X columns actually used

ALPHA = float(np.float32(np.exp(-1.0 / 20.0)))
KAPPA = float(np.float32(np.exp(-1.0 / 20.0)))
THR = 1.0


WI_OFF = 0
WI_LEN = DTILES * H  # 6144
W_OFF = WI_OFF + WI_LEN
W_LEN = KT * H  # 8192
WO_OFF = W_OFF + W_LEN
WO_LEN = KT * O  # 160
XT_OFF = WO_OFF + WO_LEN  # 14496
XT_LEN = DTILES * XCOLS  # 19008
BLOB_COLS = XT_OFF + XT_LEN  # 33504


def _build(nsteps=NSTEPS, timing=False):
    """timing=True pins the xt column offset to 0 so nsteps can exceed the
    real 99 without reading out of bounds (identical instruction mix)."""
    nc = bacc.Bacc(None)
    blob_d = nc.declare_dram_parameter("blob", [128, BLOB_COLS], F32, isOutput=False)
    vo_d = nc.declare_dram_parameter("vo16", [O, 16 * BL], F32, isOutput=True)

    with ExitStack() as ctx:
        tc = ctx.enter_context(tile.TileContext(nc))
        const = ctx.enter_context(tc.tile_pool(name="const", bufs=1))
        pp = ctx.enter_context(tc.tile_pool(name="pp", bufs=1, space="PSUM"))

        blob_sb = const.tile([128, BLOB_COLS], F32, name="blob_sb")
        xt_sb = blob_sb[:, XT_OFF : XT_OFF + XT_LEN].rearrange(
            "p (a c) -> p a c", a=DTILES
        )
        wi_sb = blob_sb[:, WI_OFF : WI_OFF + WI_LEN].rearrange(
            "p (a c) -> p a c", a=DTILES
        )
        w_sb = blob_sb[:, W_OFF : W_OFF + W_LEN].rearrange("p (a c) -> p a c", a=KT)
        wo_sb = blob_sb[:, WO_OFF : WO_OFF + WO_LEN].rearrange(
            "p (a c) -> p a c", a=KT
        )
        v = const.tile([128, KT, BL], F32, name="v")
        z = const.tile([128, KT, BL], F32, name="z")
        u = const.tile([128, KT, BL], F32, name="u")
        vo_sb = const.tile([O, 16 * BL], F32, name="vo_sb")

        # banks 0-3: scan psum (bank h//2, cols (h%2)*BL); bank 4: vo epilogue
        ps = pp.tile([128, KT, 512], F32, name="ps")

        nc.sync.dma_start(blob_sb[:], blob_d[:])
        nc.vector.memset(v[:], 0.0)
        nc.vector.memset(z[:], 0.0)
        nc.vector.memset(u[:], 0.0)
        nc.vector.memset(vo_sb[:], 0.0)

        if nsteps > 0:
            with tc.For_i(0, nsteps, 1) as t:
                xcol = 0 if timing else t * BL
                for h in range(KT):
                    out = ps[:, h // 2, (h % 2) * BL : (h % 2) * BL + BL]
                    for dk in range(DTILES):
                        w_ = 128 if dk < 5 else DLAST
                        nc.tensor.matmul(
                            out,
                            wi_sb[0:w_, dk, h * 128 : (h + 1) * 128],
                            xt_sb[0:w_, dk, ds(xcol, BL)],
                            start=(dk == 0),
                            stop=False,
                        )
                    for k in range(KT):
                        nc.tensor.matmul(
                            out,
                            w_sb[:, k, h * 128 : (h + 1) * 128],
                            z[:, k, :],
                            start=False,
                            stop=(k == KT - 1),
                        )
                # psum banks 0-3, h-pair (2p, 2p+1) at cols (0,BL) of bank p
                psum_in = ps[:, 0:4, 0 : 2 * BL].rearrange(
                    "q a (b c) -> q a b c", b=2
                )
                v4 = v.rearrange("p (a b) c -> p a b c", a=4)
                nc.vector.scalar_tensor_tensor(
                    v4[:], v4[:], ALPHA, psum_in, A.mult, A.add
                )
                nc.vector.tensor_scalar(z[:], v[:], THR, None, A.is_gt)
                nc.vector.scalar_tensor_tensor(
                    u[:], u[:], KAPPA, z[:], A.mult, A.add
                )
                vo_ps = ps[0:O, 4, 0:BL]
                for k in range(KT):
                    nc.tensor.matmul(
                        vo_ps,
                        wo_sb[:, k, :],
                        u[:, k, :],
                        start=(k == 0),
                        stop=(k == KT - 1),
                    )
                nc.vector.tensor_copy(vo_sb[:, ds((t & 15) * BL, BL)], vo_ps)
        nc.gpsimd.dma_start(vo_d[:], vo_sb[:])

    nc.compile()
    return nc


_PROGRAM = None


def _get_program():
    global _PROGRAM
    if _PROGRAM is None:
        _PROGRAM = _build()
    return _PROGRAM


def _host_prep(W_in, W_rec, W_out):
    eye = np.eye(H, dtype=np.float32)
    # z @ w_rec_eff.T - z*THR == z @ (w_rec_eff - THR*eye).T ; lhsT layout [k, h]
    WrT = (W_rec * (1.0 - eye) - np.float32(THR) * eye).T.astype(np.float32)
    WiT = np.zeros((DTILES * 128, H), np.float32)
    WiT[:D] = W_in.T.astype(np.float32)
    WoT = W_out.T.astype(np.float32)  # [H, O]
    # weight section of the blob, identical for every core: [128, cols]
    wpart = np.concatenate(
        [
            WiT.reshape(DTILES, 128, H).transpose(1, 0, 2).reshape(128, -1),
            WrT.reshape(KT, 128, H).transpose(1, 0, 2).reshape(128, -1),
            WoT.reshape(KT, 128, O).transpose(1, 0, 2).reshape(128, -1),
        ],
        axis=1,
    )
    return np.ascontiguousarray(wpart)


def kernel(X, W_in, W_rec, W_out):
    X = np.asarray(X, np.float32)
    wpart = _host_prep(
        np.asarray(W_in, np.float32), np.asarray(W_rec, np.float32),
        np.asarray(W_out, np.float32),
    )
    nc = _get_program()
    in_maps = []
    for c in range(NCORES):
        Xc = X[c * BL : (c + 1) * BL]  # [BL, T, D]
        # [D, t*BL + b] for t = 0..98 (step t uses cols t*BL:(t+1)*BL)
        XTc = np.zeros((DTILES * 128, XCOLS), np.float32)
        XTc[:D] = Xc[:, :NSTEPS, :].transpose(2, 1, 0).reshape(D, XCOLS)
        blob = np.concatenate(
            [wpart,
             XTc.reshape(DTILES, 128, XCOLS).transpose(1, 0, 2).reshape(128, -1)],
            axis=1,
        )
        in_maps.append({"blob": np.ascontiguousarray(blob)})
    res = run_bass_kernel_spmd(nc, in_maps, list(range(NCORES)))
    # vo16 per core: ring slot (t & 15) holds vo of scan step t; the last 10
    # steps 89..98 (vo_full indices 90..99) live at slots (89+s) & 15
    vo16 = np.stack([r["vo16"] for r in res.results])  # [8, O, 16*BL]
    vo16 = vo16.reshape(NCORES, O, 16, BL)
    slots = [(89 + s) & 15 for s in range(NTAIL)]
    vo = vo16[:, :, slots, :]  # [8, O, 10, BL]
    vo = vo.transpose(2, 0, 3, 1).reshape(NTAIL, B, O)
    m = vo.max(axis=2, keepdims=True)
    e = np.exp(vo - m)
    yo = e / e.sum(axis=2, keepdims=True)
    return yo.mean(axis=0).astype(np.float32)


# revision 16
# speedup vs baseline: 116.0240x; 1.8823x over previous
"""Trainium2 Bass kernel for the SRNN adapter problem (hardware-loop version).

Strategy (8 cores, data-parallel over batch B=256 -> 32 per core):
  The dominant cost in this environment is per-UNIQUE-instruction dispatch
  (~40us each, measured); looped re-execution via tc.For_i is ~free. So the
  99-step scan is expressed as hardware loops with dynamic (register-offset)
  access patterns instead of 99 unrolled copies:
    - scan loop: steps 0..98, one step per iteration; vo_t = W_out @ u_t is
      computed EVERY step (8 extra matmuls) into a 16-slot output ring at
      column (t & 15) * BL, which eliminates the snapshot + epilogue loop
      entirely (the host reads slots (89+s) & 15 for the last 10 steps)
  The input projection I = W_in @ x is hoisted out of the scan into its own
  8-chunk loop (xt streamed from DRAM, I_all resident in SBUF): its weight
  tiles load 8x total instead of 99x, and the scan body shrinks to the
  recurrence + readout (fp32 LDWEIGHTS at ~440ns/matmul is the step floor).
  Per scan step:
    psum[h] = sum_k (W_rec_eff - THR*I)T[k,h] @ z[k] (recurrence)
    v = alpha*v + psum + I[:, t] ; z = v > THR ; u = kappa*u + z  (in-place)
  Host: X pre-transposed to [D, T*BL] per core; softmax+mean over the last
  10 steps on host (0.005% of FLOPs).

All matmul arithmetic is fp32 (measured PE fp32 relnorm ~7e-8 vs fp64; the
spiking threshold makes the system chaotic, so bf16/fp32r decorrelate the
spike trains and blow the error budget).
"""

import sys

sys.path.insert(0, "/opt/trn_rl_repo")

import numpy as np
from contextlib import ExitStack

from concourse import bacc, bass, mybir, tile
from concourse.bass import ds
from concourse.bass_utils import run_bass_kernel_spmd

F32 = mybir.dt.float32
A = mybir.AluOpType

B, T, D, H, O = 256, 100, 700, 1024, 20
NCORES = 8
BL = B // NCORES  # 32 batch rows per core
KT = H // 128  # 8 k/h tiles
DTILES = 6  # ceil(700/128), last tile has 60 rows
DLAST = D - 5 * 128  # 60
NSTEPS = T - 1  # 99 scan steps
NTAIL = 10  # last-K softmax window
XCOLS = NSTEPS * BL  # 3168 transposed-X columns actually used

ALPHA = float(np.float32(np.exp(-1.0 / 20.0)))
KAPPA = float(np.float32(np.exp(-1.0 / 20.0)))
THR = 1.0


WI_OFF = 0
WI_LEN = DTILES * H  # 6144
W_OFF = WI_OFF + WI_LEN
W_LEN = KT * H  # 8192
WO_OFF = W_OFF + W_LEN
WO_LEN = KT * O  # 160
XT_OFF = WO_OFF + WO_LEN  # 14496
XT_LEN = DTILES * XCOLS  # 19008
BLOB_COLS = XT_OFF + XT_LEN  # 33504


def _build(nsteps=NSTEPS, timing=False):
    """timing=True pins the xt column offset to 0 so nsteps can exceed the
    real 99 without reading out of bounds (identical instruction mix)."""
    nc = bacc.Bacc(None)
    blob_d = nc.declare_dram_parameter("blob", [128, BLOB_COLS], F32, isOutput=False)
    vo_d = nc.declare_dram_parameter("vo16", [O, 16 * BL], F32, isOutput=True)

    CH = 396  # projection chunk: 8 * 396 = 3168 = XCOLS, fits one psum bank

    with ExitStack() as ctx:
        tc = ctx.enter_context(tile.TileContext(nc))
        const = ctx.enter_context(tc.tile_pool(name="const", bufs=1))
        pp = ctx.enter_context(tc.tile_pool(name="pp", bufs=1, space="PSUM"))

        blob_sb = const.tile([128, XT_OFF], F32, name="blob_sb")
        wi_sb = blob_sb[:, WI_OFF : WI_OFF + WI_LEN].rearrange(
            "p (a c) -> p a c", a=DTILES
        )
        w_sb = blob_sb[:, W_OFF : W_OFF + W_LEN].rearrange("p (a c) -> p a c", a=KT)
        wo_sb = blob_sb[:, WO_OFF : WO_OFF + WO_LEN].rearrange(
            "p (a c) -> p a c", a=KT
        )
        xt_dram = blob_d[:, XT_OFF : XT_OFF + XT_LEN].rearrange(
            "p (a c) -> p a c", a=DTILES
        )
        xbuf = const.tile([128, 2, DTILES, CH], F32, name="xbuf")
        i_sb = const.tile([128, KT, XCOLS], F32, name="i_sb")
        v = const.tile([128, KT, BL], F32, name="v")
        z = const.tile([128, KT, BL], F32, name="z")
        u = const.tile([128, KT, BL], F32, name="u")
        vo_sb = const.tile([O, 16 * BL], F32, name="vo_sb")

        # projection: all 8 banks; scan: banks 0-3 (bank h//2, cols
        # (h%2)*BL) + bank 4 for the vo readout
        ps = pp.tile([128, KT, 512], F32, name="ps")

        nc.sync.dma_start(blob_sb[:], blob_d[:, 0:XT_OFF])
        nc.sync.dma_start(xbuf[:, 0, :, :], xt_dram[:, :, 0:CH])
        nc.vector.memset(v[:], 0.0)
        nc.vector.memset(z[:], 0.0)
        nc.vector.memset(u[:], 0.0)
        nc.vector.memset(vo_sb[:], 0.0)

        # ---- input projection: I[h, c] = sum_dk W_inT[dk, h] @ xT[dk, c] ----
        with tc.For_i(0, 8, 1) as c:
            nc.sync.dma_start(
                xbuf[:, (c + 1) & 1, :, :],
                xt_dram[:, :, ds(((c + 1) & 7) * CH, CH)],
            )
            for h in range(KT):
                for dk in range(DTILES):
                    w_ = 128 if dk < 5 else DLAST
                    nc.tensor.matmul(
                        ps[:, h, 0:CH],
                        wi_sb[0:w_, dk, h * 128 : (h + 1) * 128],
                        xbuf[0:w_, c & 1, dk, :],
                        start=(dk == 0),
                        stop=(dk == DTILES - 1),
                    )
            nc.vector.tensor_copy(i_sb[:, :, ds(c * CH, CH)], ps[:, :, 0:CH])

        if nsteps > 0:
            with tc.For_i(0, nsteps, 1) as t:
                xcol = 0 if timing else t * BL
                for h in range(KT):
                    out = ps[:, h // 2, (h % 2) * BL : (h % 2) * BL + BL]
                    for k in range(KT):
                        nc.tensor.matmul(
                            out,
                            w_sb[:, k, h * 128 : (h + 1) * 128],
                            z[:, k, :],
                            start=(k == 0),
                            stop=(k == KT - 1),
                        )
                # psum banks 0-3, h-pair (2p, 2p+1) at cols (0,BL) of bank p
                psum_in = ps[:, 0:4, 0 : 2 * BL].rearrange(
                    "q a (b c) -> q a b c", b=2
                )
                v4 = v.rearrange("p (a b) c -> p a b c", a=4)
                nc.vector.scalar_tensor_tensor(
                    v4[:], v4[:], ALPHA, psum_in, A.mult, A.add
                )
                nc.vector.scalar_tensor_tensor(
                    v[:], v[:], 1.0, i_sb[:, :, ds(xcol, BL)], A.mult, A.add
                )
                nc.vector.tensor_scalar(z[:], v[:], THR, None, A.is_gt)
                nc.vector.scalar_tensor_tensor(
                    u[:], u[:], KAPPA, z[:], A.mult, A.add
                )
                vo_ps = ps[0:O, 4, 0:BL]
                for k in range(KT):
                    nc.tensor.matmul(
                        vo_ps,
                        wo_sb[:, k, :],
                        u[:, k, :],
                        start=(k == 0),
                        stop=(k == KT - 1),
                    )
                nc.vector.tensor_copy(vo_sb[:, ds((t & 15) * BL, BL)], vo_ps)
        nc.gpsimd.dma_start(vo_d[:], vo_sb[:])

    nc.compile()
    return nc


_PROGRAM = None


def _get_program():
    global _PROGRAM
    if _PROGRAM is None:
        _PROGRAM = _build()
    return _PROGRAM


def _host_prep(W_in, W_rec, W_out):
    eye = np.eye(H, dtype=np.float32)
    # z @ w_rec_eff.T - z*THR == z @ (w_rec_eff - THR*eye).T ; lhsT layout [k, h]
    WrT = (W_rec * (1.0 - eye) - np.float32(THR) * eye).T.astype(np.float32)
    WiT = np.zeros((DTILES * 128, H), np.float32)
    WiT[:D] = W_in.T.astype(np.float32)
    WoT = W_out.T.astype(np.float32)  # [H, O]
    # weight section of the blob, identical for every core: [128, cols]
    wpart = np.concatenate(
        [
            WiT.reshape(DTILES, 128, H).transpose(1, 0, 2).reshape(128, -1),
            WrT.reshape(KT, 128, H).transpose(1, 0, 2).reshape(128, -1),
            WoT.reshape(KT, 128, O).transpose(1, 0, 2).reshape(128, -1),
        ],
        axis=1,
    )
    return np.ascontiguousarray(wpart)


def kernel(X, W_in, W_rec, W_out):
    X = np.asarray(X, np.float32)
    wpart = _host_prep(
        np.asarray(W_in, np.float32), np.asarray(W_rec, np.float32),
        np.asarray(W_out, np.float32),
    )
    nc = _get_program()
    in_maps = []
    for c in range(NCORES):
        Xc = X[c * BL : (c + 1) * BL]  # [BL, T, D]
        # [D, t*BL + b] for t = 0..98 (step t uses cols t*BL:(t+1)*BL)
        XTc = np.zeros((DTILES * 128, XCOLS), np.float32)
        XTc[:D] = Xc[:, :NSTEPS, :].transpose(2, 1, 0).reshape(D, XCOLS)
        blob = np.concatenate(
            [wpart,
             XTc.reshape(DTILES, 128, XCOLS).transpose(1, 0, 2).reshape(128, -1)],
            axis=1,
        )
        in_maps.append({"blob": np.ascontiguousarray(blob)})
    res = run_bass_kernel_spmd(nc, in_maps, list(range(NCORES)))
    # vo16 per core: ring slot (t & 15) holds vo of scan step t; the last 10
    # steps 89..98 (vo_full indices 90..99) live at slots (89+s) & 15
    vo16 = np.stack([r["vo16"] for r in res.results])  # [8, O, 16*BL]
    vo16 = vo16.reshape(NCORES, O, 16, BL)
    slots = [(89 + s) & 15 for s in range(NTAIL)]
    vo = vo16[:, :, slots, :]  # [8, O, 10, BL]
    vo = vo.transpose(2, 0, 3, 1).reshape(NTAIL, B, O)
    m = vo.max(axis=2, keepdims=True)
    e = np.exp(vo - m)
    yo = e / e.sum(axis=2, keepdims=True)
    return yo.mean(axis=0).astype(np.float32)
